# revision 1
# baseline (speedup 1.0000x reference)
"""ConvBERT SDConv kernel for Trainium2 (8 NeuronCores, data-parallel over batch).

Problem (per core, batch element b):
  hidden -> depthwise conv (K=9) -> pointwise 768x768 (+bias) -> * query
         -> proj 768->108 (+bias) -> softmax(softmax(.)) over K
  out[s, h, d] = sum_k filt[s, h, k] * value[s + k - 4, h*64 + d]

On-chip layout strategy:
  - main chain in transposed layout [channels -> partitions, seq -> free]:
    depthwise conv = 9 accumulating diagonal matmuls (fp32r), pointwise and
    projection are plain matmuls; gating (+bias) fused into the PSUM
    evacuation with scalar_tensor_tensor; double softmax in [108, s] layout
    with k-group sums / broadcasts done by tiny indicator matmuls on PE.
  - light conv: per (head, s-tile of 120) build a [128, 120] banded weight
    matrix from filt via a diagonal-strided DMA through a DRAM bounce buffer,
    then one matmul per head against value kept in natural [s, c] layout.
    Output lands in natural [s, c] layout and DMAs straight out.

Host side (not on-device): shard over batch, transpose hidden/query,
zero-pad sequence halos, build the 9 diagonal depthwise-weight matrices and
k-group indicator matrices, cast gating/projection operands to bf16.
"""

import contextlib

import numpy as np
import ml_dtypes

import concourse.bass as bass
import concourse.bacc as bacc
import concourse.mybir as mybir
import concourse.tile as tile
from concourse.bass_utils import run_bass_kernel_spmd

BF16 = ml_dtypes.bfloat16

# problem constants (hardcoded per contest contract)
B, S, C = 8, 2048, 768
HID = 768
H, K, D = 12, 9, 64
PAD = K // 2                 # 4
NCORES = 8
P = 128                      # partitions
NCH = C // P                 # 6 channel chunks
SB = 512                     # phase-A seq block
NB = S // SB                 # 4
TILE = 120                   # phase-B seq tile (window = TILE + K - 1 = 128)
NT = (S + TILE - 1) // TILE  # 18 (last tile has 8 valid tokens)
BROW = H * TILE              # 1440 band row elements
VROWS = PAD + S + TILE       # padded value rows; covers last window

F32 = mybir.dt.float32
F32R = mybir.dt.float32r
BF = mybir.dt.bfloat16


def _build_nc(dbg=False):
    nc = bacc.Bacc(
        "TRN2",
        target_bir_lowering=False,
        debug=False,
        enable_asserts=False,
        num_devices=NCORES,
    )
    # per-core inputs
    xT = nc.dram_tensor("xT", [C, S + 2 * PAD], F32R, kind="ExternalInput")
    qT = nc.dram_tensor("qT", [C, S], BF, kind="ExternalInput")
    vp = nc.dram_tensor("vp", [VROWS, C], BF, kind="ExternalInput")
    dwdg = nc.dram_tensor("dwdg", [NCH, K, P, P], F32R, kind="ExternalInput")
    pwT = nc.dram_tensor("pwT", [HID, C], F32R, kind="ExternalInput")
    awT = nc.dram_tensor("awT", [C, H * K], BF, kind="ExternalInput")
    bias = nc.dram_tensor("bias", [C], F32, kind="ExternalInput")
    ab = nc.dram_tensor("ab", [H * K], F32, kind="ExternalInput")
    e12 = nc.dram_tensor("e12", [H * K, H], F32, kind="ExternalInput")
    e12t = nc.dram_tensor("e12t", [H, H * K], F32, kind="ExternalInput")
    out = nc.dram_tensor("out", [S, C], F32, kind="ExternalOutput")
    # distinct executable signature per kernel version — the axon/PJRT path has
    # been observed serving a stale compiled executable for same-signature builds
    ver = nc.dram_tensor("ver_salt_3", [1, 1], F32, kind="ExternalOutput")
    dbg_t = {}
    if dbg:
        dbg_t["dw"] = nc.dram_tensor("dbg_dw", [P, NCH, S], F32R, kind="ExternalOutput")
        dbg_t["ca"] = nc.dram_tensor("dbg_ca", [P, NCH, S], BF, kind="ExternalOutput")
        dbg_t["filt"] = nc.dram_tensor("dbg_filt", [H * K, S], BF, kind="ExternalOutput")
        dbg_t["band"] = nc.dram_tensor("dbg_band", [P, BROW], BF, kind="ExternalOutput")

    with tile.TileContext(nc) as tc, contextlib.ExitStack() as ctx:
        _kernel_body(tc, ctx, xT, qT, vp, dwdg, pwT, awT, bias, ab, e12, e12t, out,
                     dbg_t)
        vt = tc.nc.sbuf_tensor([1, 1], F32)
        with vt as vt_t:
            tc.nc.vector.memset(vt_t.ap(), 3.0)
            tc.nc.sync.dma_start(out=ver.ap(), in_=vt_t.ap())

    nc.compile()
    return nc


def _kernel_body(tc, ctx, xT, qT, vp, dwdg, pwT, awT, bias, ab, e12, e12t, out,
                 dbg_t={}):
    nc = tc.nc
    add = mybir.AluOpType.add
    mult = mybir.AluOpType.mult
    Exp = mybir.ActivationFunctionType.Exp

    consts = ctx.enter_context(tc.tile_pool(name="consts", bufs=1))
    filtp = ctx.enter_context(tc.tile_pool(name="filtp", bufs=1))
    dramp = ctx.enter_context(tc.tile_pool(name="dramp", bufs=1, space="DRAM"))

    # ---- constant loads (layouts: (p, chunk, ...) with p = within-chunk channel) ----
    pw_s = consts.tile([P, NCH, C], F32R)
    nc.sync.dma_start(
        out=pw_s,
        in_=bass.AP(tensor=pwT, offset=0, ap=[[C, P], [P * C, NCH], [1, C]]),
    )
    dwdg_s = consts.tile([P, NCH, K, P], F32R)
    nc.sync.dma_start(
        out=dwdg_s,
        in_=bass.AP(tensor=dwdg, offset=0, ap=[[P, P], [K * P * P, NCH], [P * P, K], [1, P]]),
    )
    aw_s = consts.tile([P, NCH, H * K], BF)
    nc.sync.dma_start(
        out=aw_s,
        in_=bass.AP(tensor=awT, offset=0, ap=[[H * K, P], [P * H * K, NCH], [1, H * K]]),
    )
    bias_s = consts.tile([P, NCH], F32)
    nc.sync.dma_start(
        out=bias_s, in_=bass.AP(tensor=bias, offset=0, ap=[[1, P], [P, NCH]])
    )
    ab_s = consts.tile([H * K, 1], F32)
    nc.sync.dma_start(out=ab_s, in_=bass.AP(tensor=ab, offset=0, ap=[[1, H * K], [0, 1]]))
    e12_s = consts.tile([H * K, H], F32)
    nc.sync.dma_start(out=e12_s, in_=e12.ap())
    e12t_s = consts.tile([H, H * K], F32)
    nc.sync.dma_start(out=e12t_s, in_=e12t.ap())

    # full-width double-softmax'd filter, bf16, [108 (h k), S]
    filt_bf = filtp.tile([H * K, S], BF)

    # band bounce buffers in DRAM (3, rotating), pre-zeroed once
    zero_s = consts.tile([P, BROW], BF)
    nc.vector.memset(zero_s, 0.0)
    band_drams = []
    zero_insts = []
    for i in range(3):
        bd = dramp.tile([P, BROW], BF, tag=f"band{i}")
        zi = nc.sync.dma_start(out=bd, in_=zero_s)
        band_drams.append(bd)
        zero_insts.append(zi)

    # ---------------- phase A: filt = smax(smax((sepconv(x)+b) * q @ awT + ab)) ----------------
    filt_writers = []
    with contextlib.ExitStack() as actx:
        xq = actx.enter_context(tc.tile_pool(name="xq", bufs=2))
        dwo = actx.enter_context(tc.tile_pool(name="dwo", bufs=2))
        cap = actx.enter_context(tc.tile_pool(name="cap", bufs=2))
        smp = actx.enter_context(tc.tile_pool(name="smp", bufs=2))
        dwps = actx.enter_context(tc.tile_pool(name="dwps", bufs=2, space="PSUM"))
        pwps = actx.enter_context(tc.tile_pool(name="pwps", bufs=2, space="PSUM"))
        atps = actx.enter_context(tc.tile_pool(name="atps", bufs=2, space="PSUM"))
        sups = actx.enter_context(tc.tile_pool(name="sups", bufs=1, space="PSUM"))
        bcps = actx.enter_context(tc.tile_pool(name="bcps", bufs=1, space="PSUM"))

        for b in range(NB):
            s0 = b * SB
            x_blk = xq.tile([P, NCH, SB + 2 * PAD], F32R)
            nc.sync.dma_start(
                out=x_blk,
                in_=bass.AP(
                    tensor=xT, offset=s0,
                    ap=[[S + 2 * PAD, P], [P * (S + 2 * PAD), NCH], [1, SB + 2 * PAD]],
                ),
            )
            q_blk = xq.tile([P, NCH, SB], BF)
            nc.sync.dma_start(
                out=q_blk,
                in_=bass.AP(tensor=qT, offset=s0, ap=[[S, P], [P * S, NCH], [1, SB]]),
            )

            # depthwise conv: 9 accumulating diagonal matmuls per channel chunk
            dw_blk = dwo.tile([P, NCH, SB], F32R)
            for c6 in range(NCH):
                dps = dwps.tile([P, SB], F32)
                for k in range(K):
                    nc.tensor.matmul(
                        dps,
                        dwdg_s[:, c6, k, :],
                        x_blk[:, c6, k:k + SB],
                        start=(k == 0), stop=(k == K - 1),
                    )
                nc.scalar.copy(out=dw_blk[:, c6, :], in_=dps)

            # pointwise matmul + fused (bias add, * query) evacuation -> bf16
            ca_blk = cap.tile([P, NCH, SB], BF)
            for cc in range(NCH):
                pps = pwps.tile([P, SB], F32)
                for hc in range(NCH):
                    nc.tensor.matmul(
                        pps,
                        pw_s[:, hc, cc * P:(cc + 1) * P],
                        dw_blk[:, hc, :],
                        start=(hc == 0), stop=(hc == NCH - 1),
                    )
                nc.vector.scalar_tensor_tensor(
                    out=ca_blk[:, cc, :],
                    in0=pps,
                    scalar=bias_s[:, cc:cc + 1],
                    in1=q_blk[:, cc, :],
                    op0=add, op1=mult,
                )

            # projection to [108, SB]
            if "dw" in dbg_t:
                nc.sync.dma_start(out=dbg_t["dw"].ap()[:, :, s0:s0 + SB], in_=dw_blk)
                nc.sync.dma_start(out=dbg_t["ca"].ap()[:, :, s0:s0 + SB], in_=ca_blk)

            aps = atps.tile([H * K, SB], F32)
            for cc in range(NCH):
                nc.tensor.matmul(
                    aps,
                    aw_s[:, cc, :],
                    ca_blk[:, cc, :],
                    start=(cc == 0), stop=(cc == NCH - 1),
                )

            # double softmax over k (9 partitions per head group)
            u1 = smp.tile([H * K, SB], F32)
            nc.scalar.activation(out=u1, in_=aps, func=Exp, bias=ab_s, scale=1.0)
            s1 = sups.tile([H, SB], F32, tag="sum")
            nc.tensor.matmul(s1, e12_s[:], u1[:], start=True, stop=True)
            r1 = smp.tile([H, SB], F32)
            nc.vector.reciprocal(out=r1, in_=s1)
            b1 = bcps.tile([H * K, SB], F32, tag="bc")
            nc.tensor.matmul(b1, e12t_s[:], r1[:], start=True, stop=True)
            p1 = smp.tile([H * K, SB], F32)
            nc.vector.tensor_mul(out=p1, in0=u1, in1=b1)

            u2 = smp.tile([H * K, SB], F32)
            nc.scalar.activation(out=u2, in_=p1, func=Exp)
            s2 = sups.tile([H, SB], F32, tag="sum")
            nc.tensor.matmul(s2, e12_s[:], u2[:], start=True, stop=True)
            r2 = smp.tile([H, SB], F32)
            nc.vector.reciprocal(out=r2, in_=s2)
            b2 = bcps.tile([H * K, SB], F32, tag="bc")
            nc.tensor.matmul(b2, e12t_s[:], r2[:], start=True, stop=True)
            fm = nc.vector.tensor_mul(out=filt_bf[:, s0:s0 + SB], in0=u2, in1=b2)
            filt_writers.append(fm)

    if "filt" in dbg_t:
        nc.sync.dma_start(out=dbg_t["filt"].ap(), in_=filt_bf)

    # ---------------- phase B: banded light conv ----------------
    with contextlib.ExitStack() as bctx:
        vtp = bctx.enter_context(tc.tile_pool(name="vtp", bufs=3))
        bsp = bctx.enter_context(tc.tile_pool(name="bsp", bufs=3))
        osp = bctx.enter_context(tc.tile_pool(name="osp", bufs=3))
        bps = bctx.enter_context(tc.tile_pool(name="bps", bufs=2, space="PSUM"))

        prev_readback = {}
        for t in range(NT):
            slen = min(TILE, S - t * TILE)
            slot = t % 3
            bd = band_drams[slot]
            # diagonal band writes: band[s + k, h*TILE + s] = filt[h*K + k, t*TILE + s]
            # Tile's dep tracker does not order these custom-AP DRAM accesses,
            # so wire the write->read->write chain explicitly.
            war_dep = prev_readback.get(slot, zero_insts[slot])
            diag_insts = []
            for h in range(H):
                di = nc.sync.dma_start(
                    out=bass.AP(
                        tensor=bd.tensor,
                        offset=bd.offset + h * TILE,
                        ap=[[BROW, K], [BROW + 1, slen]],
                    ),
                    in_=filt_bf[h * K:(h + 1) * K, t * TILE:t * TILE + slen],
                )
                tile.add_dep_helper(di.ins, war_dep.ins, reason="band WAR")
                # RAW on filt_bf: Tile misses partition-offset slice deps here
                for bb in range(t * TILE // SB, min((t * TILE + slen - 1) // SB + 1, NB)):
                    tile.add_dep_helper(di.ins, filt_writers[bb].ins, reason="filt RAW")
                diag_insts.append(di)
            band_sbuf = bsp.tile([P, BROW], BF)
            rb = nc.sync.dma_start(out=band_sbuf, in_=bd)
            for di in diag_insts:
                tile.add_dep_helper(rb.ins, di.ins, reason="band RAW")
            prev_readback[slot] = rb
            if t == 0 and "band" in dbg_t:
                nc.sync.dma_start(out=dbg_t["band"].ap(), in_=band_sbuf)
            v_sbuf = vtp.tile([P, C], BF)
            nc.sync.dma_start(out=v_sbuf, in_=vp.ap()[t * TILE:t * TILE + P, :])
            ops = bps.tile([P, C], F32)
            for h in range(H):
                nc.tensor.matmul(
                    ops[:TILE, h * D:(h + 1) * D],
                    band_sbuf[:, h * TILE:(h + 1) * TILE],
                    v_sbuf[:, h * D:(h + 1) * D],
                    start=True, stop=True,
                )
            o_sbuf = osp.tile([P, C], F32)
            nc.scalar.copy(out=o_sbuf[:slen], in_=ops[:slen])
            nc.sync.dma_start(out=out.ap()[t * TILE:t * TILE + slen, :], in_=o_sbuf[:slen])


_NC_CACHE = {}


def get_nc(dbg=False):
    if dbg not in _NC_CACHE:
        _NC_CACHE[dbg] = _build_nc(dbg)
    return _NC_CACHE[dbg]


def make_in_maps(query, value, hidden_states, dw_weight, pw_weight, sep_bias,
                 attn_W, attn_b):
    query = np.asarray(query, np.float32)
    value = np.asarray(value, np.float32)
    hidden_states = np.asarray(hidden_states, np.float32)
    dw_weight = np.asarray(dw_weight, np.float32)
    pw_weight = np.asarray(pw_weight, np.float32)
    sep_bias = np.asarray(sep_bias, np.float32)
    attn_W = np.asarray(attn_W, np.float32)
    attn_b = np.asarray(attn_b, np.float32)

    # shared (weight) tensors
    dwdg = np.zeros((NCH, K, P, P), np.float32)
    idx = np.arange(P)
    # dwdg[c6, k, i, i] = dw_weight[c6*P + i, 0, k]
    dwdg[:, :, idx, idx] = dw_weight[:, 0, :].reshape(NCH, P, K).transpose(0, 2, 1)
    pwT = np.ascontiguousarray(pw_weight[:, :, 0].T)
    awT = np.ascontiguousarray(attn_W.T).astype(BF16)
    e12 = np.repeat(np.eye(H, dtype=np.float32), K, axis=0)  # [108, 12]
    e12t = np.ascontiguousarray(e12.T)

    in_maps = []
    for b in range(NCORES):
        xT = np.zeros((C, S + 2 * PAD), np.float32)
        xT[:, PAD:PAD + S] = hidden_states[b].T
        qT = np.ascontiguousarray(query[b].T).astype(BF16)
        vpad = np.zeros((VROWS, C), BF16)
        vpad[PAD:PAD + S] = value[b].astype(BF16)
        in_maps.append({
            "xT": xT, "qT": qT, "vp": vpad,
            "dwdg": dwdg, "pwT": pwT, "awT": awT,
            "bias": sep_bias, "ab": attn_b, "e12": e12, "e12t": e12t,
        })
    return in_maps


def kernel(query, value, hidden_states, dw_weight, pw_weight, sep_bias,
           attn_W, attn_b, num_heads=None, kernel_size=None):
    # dbg=True: the extra stage-dump DMAs serialize a schedule that is
    # correct on hardware; the dbg=False schedule mis-orders the band build.
    nc = get_nc(dbg=True)
    in_maps = make_in_maps(query, value, hidden_states, dw_weight, pw_weight,
                           sep_bias, attn_W, attn_b)
    res = run_bass_kernel_spmd(nc, in_maps, core_ids=list(range(NCORES)))
    outs = [np.asarray(r["out"], np.float32) for r in res.results]
    return np.stack(outs, axis=0).reshape(B, S, H, D)



# revision 2
# speedup vs baseline: 2.8804x; 2.8804x over previous
"""ConvBERT SDConv kernel for Trainium2 (8 NeuronCores, data-parallel over batch).

Problem (per core, batch element b):
  hidden -> depthwise conv (K=9) -> pointwise 768x768 (+bias) -> * query
         -> proj 768->108 (+bias) -> softmax(softmax(.)) over K
  out[s, h, d] = sum_k filt[s, h, k] * value[s + k - 4, h*64 + d]

v2 design notes:
  - everything bf16 on the wire and in matmuls (validated ~6e-3 rel err);
    inputs preloaded whole into SBUF so the PE never DMA-starves.
  - depthwise diag weight matrices built on-chip: identity (x) per-partition
    scalar on DVE, instead of DMAing 3.5 MB of mostly-zero diagonals.
  - light conv banded matrices built WITHOUT per-element scatter DMAs:
    per 128-token tile, PE transposes a filt slice ([108, 128] -> [128, 108]),
    then 9 shift-matmuls against constant shifted identities produce
    SHR[s', h, 8-j] = filt[h*9+j, t*120+s'-j].  Each SHR row holds the 9
    band-diagonal values of band row s' for head h CONTIGUOUSLY, so ONE
    rectangular-AP DMA per tile writes the whole band into DRAM with 18-byte
    runs: band[s', h*136 + s' + jr] = SHR[s', h, jr].  Entries with invalid
    s = s' - (8-jr) land in the 8-column pads of each 136-wide head block and
    are never read.  Band zeros are written once (zero-stays-zero); readback
    is one clean rectangular DMA per tile.
  - head matmuls: lhsT = band[:, h*136+8 : h*136+136] (128 cols, FWL-able),
    moving = value rows [t*120, t*120+128) of the padded value.
"""

import contextlib

import numpy as np
import ml_dtypes

import concourse.bass as bass
import concourse.bacc as bacc
import concourse.mybir as mybir
import concourse.tile as tile
from concourse.bass_utils import run_bass_kernel_spmd

BF16 = ml_dtypes.bfloat16

# problem constants (hardcoded per contest contract)
B, S, C = 8, 2048, 768
HID = 768
H, K, D = 12, 9, 64
PAD = K // 2                 # 4
NCORES = 8
P = 128                      # partitions
NCH = C // P                 # 6 channel chunks
SB = 512                     # phase-A seq block
NB = S // SB                 # 4
TILE = 120                   # phase-B seq tile (window = TILE + K - 1 = 128)
NT = (S + TILE - 1) // TILE  # 18 (last tile has 8 valid tokens)
HB = TILE + 2 * (K - 1)      # 136: head block width = 8 pad + 120 + 8 pad
BROWP = H * HB               # 1632 band row elements (padded layout)
VROWS = PAD + S + TILE       # padded value rows; covers last window
FCOLS = (NT - 1) * TILE + P  # 2168 -> round up
FPAD = 2176                  # filt columns incl zero tail for last tile

F32 = mybir.dt.float32
F32R = mybir.dt.float32r
BF = mybir.dt.bfloat16
AF = mybir.ActivationFunctionType


def _build_nc(dbg=False):
    nc = bacc.Bacc(
        "TRN2",
        target_bir_lowering=False,
        debug=False,
        enable_asserts=False,
        num_devices=NCORES,
    )
    # per-core inputs
    xT = nc.dram_tensor("xT2", [C, S + 2 * PAD], BF, kind="ExternalInput")
    qT = nc.dram_tensor("qT2", [C, S], BF, kind="ExternalInput")
    vp = nc.dram_tensor("vp2", [VROWS, C], BF, kind="ExternalInput")
    pwT = nc.dram_tensor("pw2", [HID, C], BF, kind="ExternalInput")
    awT = nc.dram_tensor("aw2", [C, H * K], BF, kind="ExternalInput")
    dww = nc.dram_tensor("dww2", [P, NCH * K], F32, kind="ExternalInput")
    bias = nc.dram_tensor("bias2", [C], F32, kind="ExternalInput")
    ab = nc.dram_tensor("ab2", [H * K], F32, kind="ExternalInput")
    e12 = nc.dram_tensor("e12b", [H * K, H], BF, kind="ExternalInput")
    e12t = nc.dram_tensor("e12t2", [H, H * K], F32R, kind="ExternalInput")
    eye = nc.dram_tensor("eye2", [P, P], BF, kind="ExternalInput")
    shf = nc.dram_tensor("shf2", [K, P, P], BF, kind="ExternalInput")
    out = nc.dram_tensor("out", [S, C], BF, kind="ExternalOutput")
    # distinct executable signature per kernel version — the axon/PJRT path has
    # been observed serving a stale compiled executable for same-signature builds
    ver = nc.dram_tensor("ver_salt_v4", [1, 1], F32, kind="ExternalOutput")

    with tile.TileContext(nc) as tc, contextlib.ExitStack() as ctx:
        _kernel_body(tc, ctx, xT, qT, vp, pwT, awT, dww, bias, ab, e12, e12t,
                     eye, shf, out)
        vt = tc.nc.sbuf_tensor([1, 1], F32)
        with vt as vt_t:
            tc.nc.vector.memset(vt_t.ap(), 4.0)
            tc.nc.sync.dma_start(out=ver.ap(), in_=vt_t.ap())

    nc.compile()
    return nc


def _kernel_body(tc, ctx, xT, qT, vp, pwT, awT, dww, bias, ab, e12, e12t,
                 eye, shf, out):
    nc = tc.nc
    add = mybir.AluOpType.add
    mult = mybir.AluOpType.mult

    consts = ctx.enter_context(tc.tile_pool(name="consts", bufs=1))
    dramp = ctx.enter_context(tc.tile_pool(name="dramp", bufs=1, space="DRAM"))

    # ---- whole-input loads (layout: (p, chunk, ...) with p = within-chunk channel) ----
    x_sb = consts.tile([P, NCH, S + 2 * PAD], BF)
    nc.sync.dma_start(
        out=x_sb,
        in_=bass.AP(tensor=xT, offset=0,
                    ap=[[S + 2 * PAD, P], [P * (S + 2 * PAD), NCH], [1, S + 2 * PAD]]),
    )
    q_sb = consts.tile([P, NCH, S], BF)
    nc.sync.dma_start(
        out=q_sb,
        in_=bass.AP(tensor=qT, offset=0, ap=[[S, P], [P * S, NCH], [1, S]]),
    )
    pw_sb = consts.tile([P, NCH, C], BF)
    nc.sync.dma_start(
        out=pw_sb,
        in_=bass.AP(tensor=pwT, offset=0, ap=[[C, P], [P * C, NCH], [1, C]]),
    )
    aw_sb = consts.tile([P, NCH, H * K], BF)
    nc.sync.dma_start(
        out=aw_sb,
        in_=bass.AP(tensor=awT, offset=0, ap=[[H * K, P], [P * H * K, NCH], [1, H * K]]),
    )
    dww_sb = consts.tile([P, NCH * K], F32)
    nc.sync.dma_start(
        out=dww_sb,
        in_=bass.AP(tensor=dww, offset=0, ap=[[NCH * K, P], [1, NCH * K]]),
    )
    bias_sb = consts.tile([P, NCH], F32)
    nc.sync.dma_start(
        out=bias_sb, in_=bass.AP(tensor=bias, offset=0, ap=[[1, P], [P, NCH]])
    )
    ab_sb = consts.tile([H * K, 1], F32)
    nc.sync.dma_start(out=ab_sb, in_=bass.AP(tensor=ab, offset=0, ap=[[1, H * K], [0, 1]]))
    e12_sb = consts.tile([H * K, H], BF)
    nc.sync.dma_start(out=e12_sb, in_=e12.ap())
    e12t_sb = consts.tile([H, H * K], F32R)
    nc.sync.dma_start(out=e12t_sb, in_=e12t.ap())
    eye_sb = consts.tile([P, P], BF)
    nc.sync.dma_start(out=eye_sb, in_=eye.ap())
    shf_sb = consts.tile([P, K, P], BF)
    nc.sync.dma_start(
        out=shf_sb,
        in_=bass.AP(tensor=shf, offset=0, ap=[[P, P], [P * P, K], [1, P]]),
    )

    # depthwise diagonal weight matrices, built on-chip
    dwdg_sb = consts.tile([P, NCH * K, P], BF)
    for i in range(NCH * K):
        nc.vector.tensor_scalar_mul(
            out=dwdg_sb[:, i, :], in0=eye_sb, scalar1=dww_sb[:, i:i + 1]
        )

    # full-width double-softmax'd filter, bf16, [108 (h k), FPAD]
    filt_sb = consts.tile([H * K, FPAD], BF)
    nc.vector.memset(filt_sb[:, S:FPAD], 0.0)

    # band bounce buffers in DRAM (3, rotating), zeroed once; the per-tile diag
    # writes always hit the same positions, so zeros stay zero afterwards.
    zero_sb = consts.tile([P, BROWP], BF)
    nc.vector.memset(zero_sb, 0.0)
    band_drams = []
    zero_insts = []
    for i in range(3):
        bd = dramp.tile([P, BROWP], BF, tag=f"band{i}")
        zi = nc.sync.dma_start(out=bd, in_=zero_sb)
        band_drams.append(bd)
        zero_insts.append(zi)

    # ---------------- phase A: filt = smax(smax((sepconv(x)+b) * q @ awT + ab)) ----------------
    Exp = AF.Exp
    filt_writers = []
    xq = ctx.enter_context(tc.tile_pool(name="xq", bufs=2))
    dwo = ctx.enter_context(tc.tile_pool(name="dwo", bufs=2))
    cap = ctx.enter_context(tc.tile_pool(name="cap", bufs=2))
    smp = ctx.enter_context(tc.tile_pool(name="smp", bufs=2))
    dwps = ctx.enter_context(tc.tile_pool(name="dwps", bufs=1, space="PSUM"))
    pwps = ctx.enter_context(tc.tile_pool(name="pwps", bufs=1, space="PSUM"))
    abps = ctx.enter_context(tc.tile_pool(name="abps", bufs=1, space="PSUM"))
    sups = ctx.enter_context(tc.tile_pool(name="sups", bufs=1, space="PSUM"))
    # phase-B psum pools (allocated up front so A/B can interleave)
    trps = ctx.enter_context(tc.tile_pool(name="trps", bufs=1, space="PSUM"))
    shps = ctx.enter_context(tc.tile_pool(name="shps", bufs=1, space="PSUM"))
    bops = ctx.enter_context(tc.tile_pool(name="bops", bufs=1, space="PSUM"))

    for b in range(NB):
        s0 = b * SB

        # depthwise conv: 9 accumulating diagonal matmuls per channel chunk
        dw_blk = dwo.tile([P, NCH, SB], BF)
        for c6 in range(NCH):
            dps = dwps.tile([P, SB], F32, tag="dw")
            for k in range(K):
                nc.tensor.matmul(
                    dps,
                    dwdg_sb[:, c6 * K + k, :],
                    x_sb[:, c6, s0 + k:s0 + k + SB],
                    start=(k == 0), stop=(k == K - 1),
                )
            nc.scalar.copy(out=dw_blk[:, c6, :], in_=dps)

        # pointwise matmul + fused (bias add, * query) evacuation -> bf16
        ca_blk = cap.tile([P, NCH, SB], BF)
        for cc in range(NCH):
            pps = pwps.tile([P, SB], F32, tag="pw")
            for hc in range(NCH):
                nc.tensor.matmul(
                    pps,
                    pw_sb[:, hc, cc * P:(cc + 1) * P],
                    dw_blk[:, hc, :],
                    start=(hc == 0), stop=(hc == NCH - 1),
                )
            nc.vector.scalar_tensor_tensor(
                out=ca_blk[:, cc, :],
                in0=pps,
                scalar=bias_sb[:, cc:cc + 1],
                in1=q_sb[:, cc, s0:s0 + SB],
                op0=add, op1=mult,
            )

        # projection to [108, SB]
        aps = abps.tile([H * K, SB], F32, tag="ab")
        for cc in range(NCH):
            nc.tensor.matmul(
                aps,
                aw_sb[:, cc, :],
                ca_blk[:, cc, :],
                start=(cc == 0), stop=(cc == NCH - 1),
            )

        # double softmax over k (9 partitions per head group)
        u1 = smp.tile([H * K, SB], BF, tag="u1")
        nc.scalar.activation(out=u1, in_=aps, func=Exp, bias=ab_sb, scale=1.0)
        s1 = sups.tile([H, SB], F32, tag="sum")
        nc.tensor.matmul(s1, e12_sb[:], u1[:], start=True, stop=True)
        r1 = smp.tile([H, SB], F32R, tag="r1")
        with nc.allow_low_precision(reason="f32r is full fp32 precision"):
            nc.vector.reciprocal(out=r1, in_=s1)
        b1 = abps.tile([H * K, SB], F32, tag="ab")
        nc.tensor.matmul(b1, e12t_sb[:], r1[:], start=True, stop=True)
        p1 = smp.tile([H * K, SB], BF, tag="p1")
        nc.vector.tensor_mul(out=p1, in0=u1, in1=b1)

        u2 = smp.tile([H * K, SB], BF, tag="u2")
        nc.scalar.activation(out=u2, in_=p1, func=Exp)
        s2 = sups.tile([H, SB], F32, tag="sum")
        nc.tensor.matmul(s2, e12_sb[:], u2[:], start=True, stop=True)
        r2 = smp.tile([H, SB], F32R, tag="r1")
        with nc.allow_low_precision(reason="f32r is full fp32 precision"):
            nc.vector.reciprocal(out=r2, in_=s2)
        b2 = abps.tile([H * K, SB], F32, tag="ab")
        nc.tensor.matmul(b2, e12t_sb[:], r2[:], start=True, stop=True)
        fm = nc.vector.tensor_mul(out=filt_sb[:, s0:s0 + SB], in0=u2, in1=b2)
        filt_writers.append(fm)

    # ---------------- phase B: banded light conv ----------------
    tsp = ctx.enter_context(tc.tile_pool(name="tsp", bufs=2))
    srp = ctx.enter_context(tc.tile_pool(name="srp", bufs=2))
    vtp = ctx.enter_context(tc.tile_pool(name="vtp", bufs=3))
    bsp = ctx.enter_context(tc.tile_pool(name="bsp", bufs=3))
    osp = ctx.enter_context(tc.tile_pool(name="osp", bufs=2))

    prev_readback = {}
    for t in range(NT):
        slen = min(TILE, S - t * TILE)
        slot = t % 3
        bd = band_drams[slot]

        # filt slice transpose: [108, 128] -> [128, 108] (bf16, PSUM)
        t_ps = trps.tile([P, H * K], BF, tag="tr")
        nc.tensor.transpose(t_ps, filt_sb[:, t * TILE:t * TILE + P],
                            eye_sb[0:H * K, 0:H * K])
        t_sb = tsp.tile([P, H, K], BF)
        nc.scalar.copy(out=t_sb, in_=t_ps)

        # 9 shift matmuls: SHR[s', h, 8-j] = T[s'-j, h, j] = filt[h*9+j, t*120+s'-j]
        shr_ps = shps.tile([P, H, K], F32, tag="sh")
        for j in range(K):
            nc.tensor.matmul(
                shr_ps[:, :, K - 1 - j],
                shf_sb[:, j, :],
                t_sb[:, :, j],
                start=True, stop=True,
            )
        shr_sb = srp.tile([P, H, K], BF)
        nc.scalar.copy(out=shr_sb, in_=shr_ps)

        # one diag-write DMA: band[s', h*136 + s' + jr] = SHR[s', h, jr]
        # (18-byte contiguous runs; invalid entries land in the pad columns)
        di = nc.sync.dma_start(
            out=bass.AP(tensor=bd.tensor, offset=bd.offset,
                        ap=[[BROWP + 1, P], [HB, H], [1, K]]),
            in_=shr_sb,
        )
        war_dep = prev_readback.get(slot, zero_insts[slot])
        tile.add_dep_helper(di.ins, war_dep.ins, reason="band WAR")

        # clean rectangular readback
        band_sb = bsp.tile([P, BROWP], BF)
        rb = nc.sync.dma_start(
            out=band_sb,
            in_=bass.AP(tensor=bd.tensor, offset=bd.offset,
                        ap=[[BROWP, P], [1, BROWP]]),
        )
        tile.add_dep_helper(rb.ins, di.ins, reason="band RAW")
        prev_readback[slot] = rb

        v_sb = vtp.tile([P, C], BF)
        nc.sync.dma_start(out=v_sb, in_=vp.ap()[t * TILE:t * TILE + P, :])
        ops = bops.tile([P, C], F32, tag="o")
        for h in range(H):
            nc.tensor.matmul(
                ops[:, h * D:(h + 1) * D],
                band_sb[:, h * HB + K - 1:h * HB + K - 1 + P],
                v_sb[:, h * D:(h + 1) * D],
                start=True, stop=True,
            )
        o_sb = osp.tile([P, C], BF)
        nc.scalar.copy(out=o_sb[:slen], in_=ops[:slen])
        nc.sync.dma_start(out=out.ap()[t * TILE:t * TILE + slen, :], in_=o_sb[:slen])


_NC_CACHE = {}


def get_nc(dbg=False):
    if dbg not in _NC_CACHE:
        _NC_CACHE[dbg] = _build_nc(dbg)
    return _NC_CACHE[dbg]


def make_in_maps(query, value, hidden_states, dw_weight, pw_weight, sep_bias,
                 attn_W, attn_b):
    query = np.asarray(query, np.float32)
    value = np.asarray(value, np.float32)
    hidden_states = np.asarray(hidden_states, np.float32)
    dw_weight = np.asarray(dw_weight, np.float32)
    pw_weight = np.asarray(pw_weight, np.float32)
    sep_bias = np.asarray(sep_bias, np.float32)
    attn_W = np.asarray(attn_W, np.float32)
    attn_b = np.asarray(attn_b, np.float32)

    # shared (weight) tensors
    dww = np.ascontiguousarray(
        dw_weight[:, 0, :].reshape(NCH, P, K).transpose(1, 0, 2).reshape(P, NCH * K)
    )
    pwT = np.ascontiguousarray(pw_weight[:, :, 0].T).astype(BF16)
    awT = np.ascontiguousarray(attn_W.T).astype(BF16)
    e12 = np.repeat(np.eye(H, dtype=np.float32), K, axis=0)  # [108, 12]
    e12b = e12.astype(BF16)
    e12t = np.ascontiguousarray(e12.T)
    eye = np.eye(P, dtype=np.float32).astype(BF16)
    shfm = np.stack([np.eye(P, P, k=j, dtype=np.float32) for j in range(K)])
    shfm = shfm.astype(BF16)

    in_maps = []
    for b in range(NCORES):
        xTb = np.zeros((C, S + 2 * PAD), BF16)
        xTb[:, PAD:PAD + S] = hidden_states[b].T.astype(BF16)
        qTb = np.ascontiguousarray(query[b].T).astype(BF16)
        vpad = np.zeros((VROWS, C), BF16)
        vpad[PAD:PAD + S] = value[b].astype(BF16)
        in_maps.append({
            "xT2": xTb, "qT2": qTb, "vp2": vpad,
            "pw2": pwT, "aw2": awT, "dww2": dww,
            "bias2": sep_bias, "ab2": attn_b,
            "e12b": e12b, "e12t2": e12t, "eye2": eye, "shf2": shfm,
        })
    return in_maps


def kernel(query, value, hidden_states, dw_weight, pw_weight, sep_bias,
           attn_W, attn_b, num_heads=None, kernel_size=None):
    nc = get_nc()
    in_maps = make_in_maps(query, value, hidden_states, dw_weight, pw_weight,
                           sep_bias, attn_W, attn_b)
    res = run_bass_kernel_spmd(nc, in_maps, core_ids=list(range(NCORES)))
    outs = [np.asarray(r["out"]).astype(np.float32) for r in res.results]
    return np.stack(outs, axis=0).reshape(B, S, H, D)
